# revision 1
# baseline (speedup 1.0000x reference)
"""Trainium2 Bass kernel for a single-step DecoderRNN (reformat + embed+relu +
LSTM cell + vocab output projection), sharded over 8 NeuronCores.

Sharding: each core m owns hidden indices [128m, 128m+128). It computes its
slice of the LSTM state update and a partial [1, V] logits contribution from
its 128 rows of the contraction dim of W_out; the host sums the 8 partials.
No cross-core communication is needed on device.

Host-side folding: h = hidden @ W_ref.T + b_ref only feeds the gates through
W_hh, so W_hh @ W_ref is precomputed on host and the gate matvec contracts
directly against concat(relu(emb[tok]), hidden).
"""
import numpy as np

H = 1024
V = 50257
N_CORES = 8
VPAD = 51200          # 400 vocab tiles of 128
NVT = VPAD // 128     # 400
KZ = 2176             # gate contraction: 1024 (x) + 1028 (hidden) padded to 17*128
NT_Z = KZ // 128      # 17
KC = 1152             # cell contraction: 1028 padded to 9*128
NT_C = KC // 128      # 9
VCHUNK = 6400         # vocab columns per W DMA chunk (50 tiles, 3.27 MB)
NCHUNK = VPAD // VCHUNK

_cache = {}


def _build_bass():
    import concourse.bacc as bacc
    import concourse.bass as bass
    from concourse import mybir, tile

    f32 = mybir.dt.float32
    AF = mybir.ActivationFunctionType

    nc = bacc.Bacc("TRN2", target_bir_lowering=False, debug=False,
                   num_devices=N_CORES)

    z_d = nc.dram_tensor("z", [128, NT_Z], f32, kind="ExternalInput")
    cz_d = nc.dram_tensor("cz", [128, NT_C], f32, kind="ExternalInput")
    g_d = nc.dram_tensor("g_w", [KZ, 512], f32, kind="ExternalInput")
    ac_d = nc.dram_tensor("ac_w", [KC, 128], f32, kind="ExternalInput")
    bias_d = nc.dram_tensor("bias", [128, 4], f32, kind="ExternalInput")
    bref_d = nc.dram_tensor("bref", [128, 1], f32, kind="ExternalInput")
    wt_d = nc.dram_tensor("wt", [128, VPAD], f32, kind="ExternalInput")
    oh_d = nc.dram_tensor("out_h", [128, 1], f32, kind="ExternalOutput")
    oc_d = nc.dram_tensor("out_c", [128, 1], f32, kind="ExternalOutput")
    op_d = nc.dram_tensor("out_part", [128, NVT], f32, kind="ExternalOutput")

    with tile.TileContext(nc) as tc:
        with (
            tc.tile_pool(name="const", bufs=1) as cpool,
            tc.tile_pool(name="wpool", bufs=4) as wpool,
            tc.tile_pool(name="psum", bufs=1, space=bass.MemorySpace.PSUM) as ppool,
        ):
            z_sb = cpool.tile([128, NT_Z], f32)
            nc.sync.dma_start(z_sb[:], z_d.ap())
            # x part (k < 1024 -> columns 0..7) gets the embedding relu
            nc.scalar.activation(z_sb[:, 0:8], z_sb[:, 0:8], AF.Relu)

            cz_sb = cpool.tile([128, NT_C], f32)
            nc.sync.dma_start(cz_sb[:], cz_d.ap())

            g_sb = cpool.tile([128, NT_Z, 512], f32)
            nc.sync.dma_start(g_sb[:], g_d.ap().rearrange("(t p) j -> p t j", p=128))

            ac_sb = cpool.tile([128, NT_C, 128], f32)
            nc.sync.dma_start(ac_sb[:], ac_d.ap().rearrange("(t p) j -> p t j", p=128))

            bias_sb = cpool.tile([128, 4], f32)
            nc.sync.dma_start(bias_sb[:], bias_d.ap())
            bref_sb = cpool.tile([128, 1], f32)
            nc.sync.dma_start(bref_sb[:], bref_d.ap())

            # gates[p, g] = sum_k z[k] * G[k, g*128+p]   (i, f, g, o blocks)
            psum_g = ppool.tile([128, 4], f32)
            for g in range(4):
                for t in range(NT_Z):
                    nc.tensor.matmul(
                        psum_g[:, g:g + 1],
                        g_sb[:, t, g * 128:(g + 1) * 128],
                        z_sb[:, t:t + 1],
                        start=(t == 0), stop=(t == NT_Z - 1),
                    )

            # c_reformat[p] = sum_k cell[k] * W_ref[128m+p, k]
            psum_c = ppool.tile([128, 1], f32)
            for t in range(NT_C):
                nc.tensor.matmul(
                    psum_c[:, 0:1],
                    ac_sb[:, t, :],
                    cz_sb[:, t:t + 1],
                    start=(t == 0), stop=(t == NT_C - 1),
                )

            si = cpool.tile([128, 1], f32)
            nc.scalar.activation(si[:], psum_g[:, 0:1], AF.Sigmoid, bias=bias_sb[:, 0:1])
            sf = cpool.tile([128, 1], f32)
            nc.scalar.activation(sf[:], psum_g[:, 1:2], AF.Sigmoid, bias=bias_sb[:, 1:2])
            tg = cpool.tile([128, 1], f32)
            nc.scalar.activation(tg[:], psum_g[:, 2:3], AF.Tanh, bias=bias_sb[:, 2:3])
            so = cpool.tile([128, 1], f32)
            nc.scalar.activation(so[:], psum_g[:, 3:4], AF.Sigmoid, bias=bias_sb[:, 3:4])

            c_in = cpool.tile([128, 1], f32)
            nc.vector.tensor_add(c_in[:], psum_c[:, 0:1], bref_sb[:])
            m1 = cpool.tile([128, 1], f32)
            nc.vector.tensor_mul(m1[:], sf[:], c_in[:])
            c_new = cpool.tile([128, 1], f32)
            # c_new = (tg * si) + m1
            nc.vector.scalar_tensor_tensor(
                c_new[:], tg[:], si[:], m1[:],
                mybir.AluOpType.mult, mybir.AluOpType.add,
            )
            tcn = cpool.tile([128, 1], f32)
            nc.scalar.activation(tcn[:], c_new[:], AF.Tanh)
            h_new = cpool.tile([128, 1], f32)
            nc.vector.tensor_mul(h_new[:], so[:], tcn[:])

            nc.sync.dma_start(oh_d.ap(), h_new[:])
            nc.sync.dma_start(oc_d.ap(), c_new[:])

            # partial logits: out[p, j] = sum_k h_new[k] * W_out[128j+p, 128m+k]
            psum_o = ppool.tile([128, NVT], f32)
            tiles_per_chunk = VCHUNK // 128
            for ch in range(NCHUNK):
                w_sb = wpool.tile([128, VCHUNK], f32)
                nc.sync.dma_start(w_sb[:], wt_d.ap()[:, ch * VCHUNK:(ch + 1) * VCHUNK])
                for j in range(tiles_per_chunk):
                    col = ch * tiles_per_chunk + j
                    nc.tensor.matmul(
                        psum_o[:, col:col + 1],
                        w_sb[:, j * 128:(j + 1) * 128],
                        h_new[:, 0:1],
                        start=True, stop=True,
                    )

            out_sb = cpool.tile([128, NVT], f32)
            nc.vector.tensor_copy(out_sb[:], psum_o[:])
            nc.sync.dma_start(op_d.ap(), out_sb[:])

    nc.finalize()
    return nc


def _prep_in_maps(input, hidden, cell, W_ref, b_ref, emb, W_ih, W_hh, b_ih,
                  b_hh, W_out, b_out):
    tok = int(np.asarray(input).reshape(-1)[0])
    x_row = np.asarray(emb[tok], dtype=np.float32).reshape(H)
    hidden_flat = np.asarray(hidden, dtype=np.float32).reshape(-1)
    cell_flat = np.asarray(cell, dtype=np.float32).reshape(-1)
    W_ref = np.asarray(W_ref, dtype=np.float32)
    b_ref = np.asarray(b_ref, dtype=np.float32)
    W_ih = np.asarray(W_ih, dtype=np.float32)
    W_hh = np.asarray(W_hh, dtype=np.float32)
    b_ih = np.asarray(b_ih, dtype=np.float32)
    b_hh = np.asarray(b_hh, dtype=np.float32)
    W_out = np.asarray(W_out, dtype=np.float32)
    KH = W_ref.shape[1]  # 1028

    W_hh_eff = W_hh @ W_ref                      # [4H, KH]
    bias_eff = b_ih + b_hh + W_hh @ b_ref        # [4H]

    z_raw = np.zeros(KZ, np.float32)
    z_raw[:H] = x_row
    z_raw[H:H + KH] = hidden_flat
    z_host = np.ascontiguousarray(z_raw.reshape(NT_Z, 128).T)     # [128, 17]

    cz_raw = np.zeros(KC, np.float32)
    cz_raw[:KH] = cell_flat
    cz_host = np.ascontiguousarray(cz_raw.reshape(NT_C, 128).T)   # [128, 9]

    WT = np.ascontiguousarray(W_out.T)           # [H, V]

    in_maps = []
    for m in range(N_CORES):
        sl = slice(128 * m, 128 * (m + 1))
        rows = (np.arange(4)[:, None] * H + 128 * m + np.arange(128)[None, :]
                ).reshape(-1)                    # [512]
        G = np.zeros((KZ, 512), np.float32)
        G[:H] = W_ih[rows].T
        G[H:H + KH] = W_hh_eff[rows].T
        Ac = np.zeros((KC, 128), np.float32)
        Ac[:KH] = W_ref[sl].T
        bias_m = np.ascontiguousarray(bias_eff[rows].reshape(4, 128).T)  # [128,4]
        bref_m = np.ascontiguousarray(b_ref[sl].reshape(128, 1))
        Wt = np.zeros((128, VPAD), np.float32)
        Wt[:, :V] = WT[sl]
        in_maps.append({
            "z": z_host, "cz": cz_host, "g_w": G, "ac_w": Ac,
            "bias": bias_m, "bref": bref_m, "wt": Wt,
        })
    return in_maps


def _assemble(results, b_out):
    b_out = np.asarray(b_out, dtype=np.float32)
    total = np.zeros((128, NVT), np.float32)
    h_parts, c_parts = [], []
    for r in results:
        total += r["out_part"]
        h_parts.append(r["out_h"].reshape(128))
        c_parts.append(r["out_c"].reshape(128))
    logits = np.ascontiguousarray(total.T).reshape(-1)[:V] + b_out
    output = logits.reshape(1, V).astype(np.float32)
    h_new = np.concatenate(h_parts).reshape(1, 1, H).astype(np.float32)
    c_new = np.concatenate(c_parts).reshape(1, 1, H).astype(np.float32)
    return output, h_new, c_new


def _kernel_impl(inputs, trace=False, trace_cores=None):
    from concourse.bass_utils import run_bass_kernel_spmd

    if "nc" not in _cache:
        _cache["nc"] = _build_bass()
    nc = _cache["nc"]
    in_maps = _prep_in_maps(**inputs)
    res = run_bass_kernel_spmd(nc, in_maps, list(range(N_CORES)),
                               trace=trace, trace_cores=trace_cores)
    outs = _assemble(res.results, inputs["b_out"])
    return outs, res


def kernel(**inputs):
    outs, _ = _kernel_impl(inputs, trace=False)
    return outs


# revision 4
# speedup vs baseline: 1.4118x; 1.4118x over previous
"""Trainium2 Bass kernel for a single-step DecoderRNN (reformat + embed+relu +
LSTM cell + vocab output projection), sharded over 8 NeuronCores.

Sharding: each core m owns hidden indices [128m, 128m+128). It computes its
slice of the LSTM state update and a partial [1, V] logits contribution from
its 128 columns of W_out (contraction dim); the host sums the 8 partials.
No cross-core communication on device.

Host-side folding: h = hidden @ W_ref.T + b_ref only feeds the gates through
W_hh, so W_hh @ W_ref is precomputed on host and the gate matvec contracts
directly against concat(relu(emb[tok]), hidden).

Compute strategy (v2): all matvecs run on the Vector engine in exact fp32 —
natural weight layout (output rows on partitions, contraction along the free
dim), the vector operand broadcast across partitions, fused multiply+reduce
(tensor_tensor_reduce for the LSTM, batched tensor_mul + segmented
tensor_reduce for the 205 MB W_out stream). The fp32 TensorE path costs
~430 ns per 128x128 tile (self-loading LDWEIGHTS), which made v1 PE-bound
at 256 us; DVE does the same tile in ~100 ns of streamed cycles.
"""
import numpy as np

H = 1024
V = 50257
N_CORES = 8
VPAD = 51200          # 400 vocab tiles of 128
NVT = VPAD // 128     # 400
KZ = 2176             # gate contraction: 1024 (x) + 1028 (hidden) padded
KC = 1152             # cell contraction: 1028 padded
VCHUNK = 5120         # vocab rows per W DMA chunk (40 tiles, 2.62 MB)
NCHUNK = VPAD // VCHUNK
TPC = VCHUNK // 128   # tiles per chunk

_cache = {}


def _build_bass():
    import concourse.bacc as bacc
    import concourse.bass as bass
    from concourse import mybir, tile

    f32 = mybir.dt.float32
    AF = mybir.ActivationFunctionType
    ALU = mybir.AluOpType

    nc = bacc.Bacc("TRN2", target_bir_lowering=False, debug=False,
                   num_devices=N_CORES)

    zb_d = nc.dram_tensor("zb", [128, KZ], f32, kind="ExternalInput")
    cb_d = nc.dram_tensor("cb", [128, KC], f32, kind="ExternalInput")
    gn_d = nc.dram_tensor("gn", [512, KZ], f32, kind="ExternalInput")
    acn_d = nc.dram_tensor("acn", [128, KC], f32, kind="ExternalInput")
    bias_d = nc.dram_tensor("bias", [128, 4], f32, kind="ExternalInput")
    bref_d = nc.dram_tensor("bref", [128, 1], f32, kind="ExternalInput")
    wn_d = nc.dram_tensor("wn", [VPAD, 128], f32, kind="ExternalInput")
    oh_d = nc.dram_tensor("out_h", [128, 1], f32, kind="ExternalOutput")
    oc_d = nc.dram_tensor("out_c", [128, 1], f32, kind="ExternalOutput")
    op_d = nc.dram_tensor("out_part", [128, NVT], f32, kind="ExternalOutput")

    with tile.TileContext(nc) as tc:
        with (
            tc.tile_pool(name="const", bufs=1) as cpool,
            tc.tile_pool(name="wpool", bufs=3) as wpool,
            tc.tile_pool(name="scr", bufs=2) as spool,
            tc.tile_pool(name="dram", bufs=1, space="DRAM") as dpool,
        ):
            zb_sb = cpool.tile([128, KZ], f32)
            nc.sync.dma_start(zb_sb[:], zb_d.ap())
            # x part (k < 1024) is the raw embedding row: apply relu
            nc.scalar.activation(zb_sb[:, 0:H], zb_sb[:, 0:H], AF.Relu)

            cb_sb = cpool.tile([128, KC], f32)
            nc.sync.dma_start(cb_sb[:], cb_d.ap())
            gn_sb = cpool.tile([128, 4, KZ], f32)
            nc.sync.dma_start(gn_sb[:], gn_d.ap().rearrange("(g p) k -> p g k", p=128))
            acn_sb = cpool.tile([128, KC], f32)
            nc.sync.dma_start(acn_sb[:], acn_d.ap())
            bias_sb = cpool.tile([128, 4], f32)
            nc.sync.dma_start(bias_sb[:], bias_d.ap())
            bref_sb = cpool.tile([128, 1], f32)
            nc.sync.dma_start(bref_sb[:], bref_d.ap())

            # gates[p] = bias[p] + sum_k G[p_row, k] * z[k]   (i, f, g, o)
            # (tensor_tensor_reduce faults on this hw path; use mul+reduce)
            zb_bc2 = zb_sb[:].rearrange("p (c k) -> p c k", c=1).broadcast_to(
                [128, 2, KZ])
            gacc0 = cpool.tile([128, 4], f32)
            for half in range(2):
                scr = spool.tile([128, VCHUNK], f32, tag="scr")
                m3 = scr[:, 0:2 * KZ].rearrange("p (c k) -> p c k", k=KZ)
                nc.vector.tensor_mul(m3, gn_sb[:, 2 * half:2 * half + 2, :], zb_bc2)
                nc.vector.tensor_reduce(
                    gacc0[:, 2 * half:2 * half + 2], m3,
                    mybir.AxisListType.X, ALU.add,
                )
            gacc = cpool.tile([128, 4], f32)
            nc.vector.tensor_add(gacc[:], gacc0[:], bias_sb[:])
            # c_reformat[p] = bref[p] + sum_k W_ref[row, k] * cell[k]
            scr = spool.tile([128, VCHUNK], f32, tag="scr")
            nc.vector.tensor_mul(scr[:, 0:KC], acn_sb[:], cb_sb[:])
            c_in0 = cpool.tile([128, 1], f32)
            nc.vector.tensor_reduce(
                c_in0[:], scr[:, 0:KC], mybir.AxisListType.X, ALU.add)
            c_in = cpool.tile([128, 1], f32)
            nc.vector.tensor_add(c_in[:], c_in0[:], bref_sb[:])

            si = cpool.tile([128, 1], f32)
            nc.scalar.activation(si[:], gacc[:, 0:1], AF.Sigmoid)
            sf = cpool.tile([128, 1], f32)
            nc.scalar.activation(sf[:], gacc[:, 1:2], AF.Sigmoid)
            tg = cpool.tile([128, 1], f32)
            nc.scalar.activation(tg[:], gacc[:, 2:3], AF.Tanh)
            so = cpool.tile([128, 1], f32)
            nc.scalar.activation(so[:], gacc[:, 3:4], AF.Sigmoid)

            m1 = cpool.tile([128, 1], f32)
            nc.vector.tensor_mul(m1[:], sf[:], c_in[:])
            c_new = cpool.tile([128, 1], f32)
            nc.vector.scalar_tensor_tensor(
                c_new[:], tg[:], si[:], m1[:], ALU.mult, ALU.add)
            tcn = cpool.tile([128, 1], f32)
            nc.scalar.activation(tcn[:], c_new[:], AF.Tanh)
            h_new = cpool.tile([128, 1], f32)
            nc.vector.tensor_mul(h_new[:], so[:], tcn[:])

            nc.sync.dma_start(oh_d.ap(), h_new[:])
            nc.sync.dma_start(oc_d.ap(), c_new[:])

            # broadcast h across partitions via a DRAM round trip
            h_rt = dpool.tile([128, 1], f32)
            nc.sync.dma_start(h_rt[:], h_new[:])
            hb_sb = cpool.tile([128, 128], f32)
            nc.sync.dma_start(
                hb_sb[:],
                h_rt[:].rearrange("p one -> one p").broadcast_to([128, 128]),
            )
            hb_bc = hb_sb[:].rearrange("p (c k) -> p c k", c=1).broadcast_to(
                [128, TPC, 128])

            # partial logits: out[p, j] = sum_k W_out[128j+p, 128m+k] * h[k]
            out_sb = cpool.tile([128, NVT], f32)
            for ch in range(NCHUNK):
                w_sb = wpool.tile([128, TPC, 128], f32)
                nc.sync.dma_start(
                    w_sb[:],
                    wn_d.ap()[ch * VCHUNK:(ch + 1) * VCHUNK, :]
                    .rearrange("(c p) k -> p c k", p=128),
                )
                mscr = spool.tile([128, VCHUNK], f32, tag="scr")
                m3 = mscr[:].rearrange("p (c k) -> p c k", k=128)
                nc.vector.tensor_mul(m3, w_sb[:], hb_bc)
                nc.vector.tensor_reduce(
                    out_sb[:, ch * TPC:(ch + 1) * TPC], m3,
                    mybir.AxisListType.X, mybir.AluOpType.add,
                )

            nc.sync.dma_start(op_d.ap(), out_sb[:])

    nc.finalize()
    return nc


def _prep_in_maps(input, hidden, cell, W_ref, b_ref, emb, W_ih, W_hh, b_ih,
                  b_hh, W_out, b_out):
    tok = int(np.asarray(input).reshape(-1)[0])
    x_row = np.asarray(emb[tok], dtype=np.float32).reshape(H)
    hidden_flat = np.asarray(hidden, dtype=np.float32).reshape(-1)
    cell_flat = np.asarray(cell, dtype=np.float32).reshape(-1)
    W_ref = np.asarray(W_ref, dtype=np.float32)
    b_ref = np.asarray(b_ref, dtype=np.float32)
    W_ih = np.asarray(W_ih, dtype=np.float32)
    W_hh = np.asarray(W_hh, dtype=np.float32)
    b_ih = np.asarray(b_ih, dtype=np.float32)
    b_hh = np.asarray(b_hh, dtype=np.float32)
    W_out = np.asarray(W_out, dtype=np.float32)
    KH = W_ref.shape[1]  # 1028

    W_hh_eff = W_hh @ W_ref                      # [4H, KH]
    bias_eff = b_ih + b_hh + W_hh @ b_ref        # [4H]

    z_raw = np.zeros(KZ, np.float32)
    z_raw[:H] = x_row
    z_raw[H:H + KH] = hidden_flat
    zb = np.ascontiguousarray(np.broadcast_to(z_raw, (128, KZ)))

    cz_raw = np.zeros(KC, np.float32)
    cz_raw[:KH] = cell_flat
    cb = np.ascontiguousarray(np.broadcast_to(cz_raw, (128, KC)))

    in_maps = []
    for m in range(N_CORES):
        sl = slice(128 * m, 128 * (m + 1))
        rows = (np.arange(4)[:, None] * H + 128 * m + np.arange(128)[None, :]
                ).reshape(-1)                    # [512]
        Gn = np.zeros((512, KZ), np.float32)
        Gn[:, :H] = W_ih[rows]
        Gn[:, H:H + KH] = W_hh_eff[rows]
        Acn = np.zeros((128, KC), np.float32)
        Acn[:, :KH] = W_ref[sl]
        bias_m = np.ascontiguousarray(bias_eff[rows].reshape(4, 128).T)  # [128,4]
        bref_m = np.ascontiguousarray(b_ref[sl].reshape(128, 1))
        Wn = np.zeros((VPAD, 128), np.float32)
        Wn[:V] = W_out[:, sl]
        in_maps.append({
            "zb": zb, "cb": cb, "gn": Gn, "acn": Acn,
            "bias": bias_m, "bref": bref_m, "wn": Wn,
        })
    return in_maps


def _assemble(results, b_out):
    b_out = np.asarray(b_out, dtype=np.float32)
    total = np.zeros((128, NVT), np.float32)
    h_parts, c_parts = [], []
    for r in results:
        total += r["out_part"]
        h_parts.append(r["out_h"].reshape(128))
        c_parts.append(r["out_c"].reshape(128))
    logits = np.ascontiguousarray(total.T).reshape(-1)[:V] + b_out
    output = logits.reshape(1, V).astype(np.float32)
    h_new = np.concatenate(h_parts).reshape(1, 1, H).astype(np.float32)
    c_new = np.concatenate(c_parts).reshape(1, 1, H).astype(np.float32)
    return output, h_new, c_new


def _kernel_impl(inputs, trace=False, trace_cores=None):
    from concourse.bass_utils import run_bass_kernel_spmd

    if "nc" not in _cache:
        _cache["nc"] = _build_bass()
    nc = _cache["nc"]
    in_maps = _prep_in_maps(**inputs)
    res = run_bass_kernel_spmd(nc, in_maps, list(range(N_CORES)),
                               trace=trace, trace_cores=trace_cores)
    outs = _assemble(res.results, inputs["b_out"])
    return outs, res


def kernel(**inputs):
    outs, _ = _kernel_impl(inputs, trace=False)
    return outs


# revision 7
# speedup vs baseline: 2.0539x; 1.4548x over previous
"""Trainium2 Bass kernel for a single-step DecoderRNN (reformat + embed+relu +
LSTM cell + vocab output projection), sharded over 8 NeuronCores.

Sharding: each core m owns hidden indices [128m, 128m+128). It computes its
slice of the LSTM state update and a partial [1, V] logits contribution from
its 128 columns of W_out (contraction dim); the host sums the 8 partials.
No cross-core communication on device.

Host-side folding: h = hidden @ W_ref.T + b_ref only feeds the gates through
W_hh, so W_hh @ W_ref is precomputed on host and the gate matvec contracts
directly against concat(relu(emb[tok]), hidden).

Compute strategy (v3):
- LSTM gates + cell reformat: PE streaming matmuls (stationary = z column
  tile [128,1], moving = weight tile [128,<=512]), outputs as [1,512]/[1,128]
  rows; elementwise LSTM on 1-lane row slices (tiny).
- W_out partial logits split across two engines so both hide under the DMA
  stream: vocab [0, VP_PE) on PE (fp32 streaming, exact), vocab [VP_PE, VPAD)
  on DVE (natural layout, broadcast-h tensor_mul + segmented tensor_reduce).
- All bulk weight DMAs go on the sync HWDGE ring in priority order (LSTM
  weights first, then W chunks interleaved PE/DVE); small latency-critical
  DMAs (h round-trip/broadcast, psum evacuations) use the GpSimd SWDGE ring
  so they never queue behind multi-MB transfers.
"""
import numpy as np

H = 1024
V = 50257
N_CORES = 8
VPAD = 51200          # 400 vocab tiles of 128
KZ = 2176             # gate contraction: 1024 (x) + 1028 (hidden) padded
NT_Z = KZ // 128      # 17
KC = 1152             # cell contraction: 1028 padded
NT_C = KC // 128      # 9
VP_PE = 30720         # vocab handled by TensorE (60 x 512)
VP_DVE = VPAD - VP_PE # 20480 vocab on VectorE (4 chunks x 40 tiles x 128)
VCHUNK = 5120
N_WT = VP_PE // VCHUNK   # 6 PE weight chunks
N_WN = VP_DVE // VCHUNK  # 4 DVE weight chunks
TPC = VCHUNK // 128      # 40 tiles per DVE chunk
NVT_DVE = VP_DVE // 128  # 160
N_PAIR = VP_PE // 1024   # 30 psum evac pairs

_cache = {}


def _build_bass():
    import concourse.bacc as bacc
    import concourse.bass as bass
    from concourse import mybir, tile

    f32 = mybir.dt.float32
    AF = mybir.ActivationFunctionType
    ALU = mybir.AluOpType

    nc = bacc.Bacc("TRN2", target_bir_lowering=False, debug=False,
                   num_devices=N_CORES)

    z_d = nc.dram_tensor("z", [128, NT_Z], f32, kind="ExternalInput")
    cz_d = nc.dram_tensor("cz", [128, NT_C], f32, kind="ExternalInput")
    g_d = nc.dram_tensor("g_w", [KZ, 512], f32, kind="ExternalInput")
    ac_d = nc.dram_tensor("ac_w", [KC, 128], f32, kind="ExternalInput")
    bias_d = nc.dram_tensor("bias", [1, 512], f32, kind="ExternalInput")
    bref_d = nc.dram_tensor("bref", [1, 128], f32, kind="ExternalInput")
    wt_d = nc.dram_tensor("wt", [128, VP_PE], f32, kind="ExternalInput")
    wn_d = nc.dram_tensor("wn", [VP_DVE, 128], f32, kind="ExternalInput")
    oh_d = nc.dram_tensor("out_h", [1, 128], f32, kind="ExternalOutput")
    oc_d = nc.dram_tensor("out_c", [1, 128], f32, kind="ExternalOutput")
    ope_d = nc.dram_tensor("out_pe", [N_PAIR, 1024], f32, kind="ExternalOutput")
    opd_d = nc.dram_tensor("out_dve", [128, NVT_DVE], f32, kind="ExternalOutput")

    with tile.TileContext(nc) as tc:
        with (
            tc.tile_pool(name="const", bufs=1) as cpool,
            tc.tile_pool(name="wtp", bufs=3) as wtpool,
            tc.tile_pool(name="wnp", bufs=2) as wnpool,
            tc.tile_pool(name="scr", bufs=1) as spool,
            tc.tile_pool(name="row", bufs=1) as rpool,
            tc.tile_pool(name="evac", bufs=3) as epool,
            tc.tile_pool(name="ps", bufs=2, space=bass.MemorySpace.PSUM) as pspool,
            tc.tile_pool(name="psl", bufs=1, space=bass.MemorySpace.PSUM) as pslpool,
            tc.tile_pool(name="dram", bufs=1, space="DRAM") as dpool,
        ):
            # ---- LSTM inputs (first in the sync FIFO = highest priority) ----
            z_sb = cpool.tile([128, NT_Z], f32)
            nc.sync.dma_start(z_sb[:], z_d.ap())
            nc.scalar.activation(z_sb[:, 0:8], z_sb[:, 0:8], AF.Relu)
            cz_sb = cpool.tile([128, NT_C], f32)
            nc.sync.dma_start(cz_sb[:], cz_d.ap())
            ac_sb = cpool.tile([128, NT_C, 128], f32)
            nc.sync.dma_start(ac_sb[:], ac_d.ap().rearrange("(t p) j -> p t j", p=128))
            bias_sb = cpool.tile([1, 512], f32)
            nc.sync.dma_start(bias_sb[:], bias_d.ap())
            bref_sb = cpool.tile([1, 128], f32)
            nc.sync.dma_start(bref_sb[:], bref_d.ap())
            # gate weights in 4 slabs so matmuls pipeline with the transfer
            g_sb = cpool.tile([128, NT_Z, 512], f32)
            slabs = [(0, 5), (5, 9), (9, 13), (13, 17)]
            for a, b in slabs:
                nc.sync.dma_start(
                    g_sb[:, a:b, :],
                    g_d.ap()[a * 128:b * 128, :].rearrange("(t p) j -> p t j", p=128))

            # ---- gates: [1,512] = sum_t z_t^T @ G_t ----
            psum_g = pslpool.tile([1, 512], f32, tag="pg")
            for t in range(NT_Z):
                nc.tensor.matmul(psum_g[:], z_sb[:, t:t + 1], g_sb[:, t, :],
                                 start=(t == 0), stop=(t == NT_Z - 1))
            # ---- cell reformat: [1,128] ----
            psum_c = pslpool.tile([1, 128], f32, tag="pc")
            for t in range(NT_C):
                nc.tensor.matmul(psum_c[:], cz_sb[:, t:t + 1], ac_sb[:, t, :],
                                 start=(t == 0), stop=(t == NT_C - 1))

            # ---- elementwise LSTM on row layout ----
            gr = rpool.tile([1, 512], f32, tag="gr")
            nc.vector.tensor_add(gr[:], psum_g[:], bias_sb[:])
            si = rpool.tile([1, 128], f32, tag="si")
            nc.scalar.activation(si[:], gr[:, 0:128], AF.Sigmoid)
            sf = rpool.tile([1, 128], f32, tag="sf")
            nc.scalar.activation(sf[:], gr[:, 128:256], AF.Sigmoid)
            tg = rpool.tile([1, 128], f32, tag="tg")
            nc.scalar.activation(tg[:], gr[:, 256:384], AF.Tanh)
            so = rpool.tile([1, 128], f32, tag="so")
            nc.scalar.activation(so[:], gr[:, 384:512], AF.Sigmoid)
            c_in = rpool.tile([1, 128], f32, tag="ci")
            nc.vector.tensor_add(c_in[:], psum_c[:], bref_sb[:])
            m1 = rpool.tile([1, 128], f32, tag="m1")
            nc.vector.tensor_mul(m1[:], sf[:], c_in[:])
            c_new = rpool.tile([1, 128], f32, tag="cn")
            nc.vector.tensor_mul(c_new[:], si[:], tg[:])
            nc.vector.tensor_add(c_new[:], c_new[:], m1[:])
            tcn = rpool.tile([1, 128], f32, tag="tc")
            nc.scalar.activation(tcn[:], c_new[:], AF.Tanh)
            h_row = rpool.tile([1, 128], f32, tag="hr")
            nc.vector.tensor_mul(h_row[:], so[:], tcn[:])

            nc.gpsimd.dma_start(oh_d.ap(), h_row[:])
            nc.gpsimd.dma_start(oc_d.ap(), c_new[:])

            # h round trip: column view for PE, partition-broadcast for DVE
            h_rt = dpool.tile([1, 128], f32)
            nc.gpsimd.dma_start(h_rt[:], h_row[:])
            h_col = cpool.tile([128, 1], f32)
            nc.gpsimd.dma_start(h_col[:], h_rt[:].rearrange("one p -> p one"))
            hb_sb = cpool.tile([128, 128], f32)
            nc.gpsimd.dma_start(hb_sb[:], h_rt[:].broadcast_to([128, 128]))
            hb_bc = hb_sb[:].rearrange("p (c k) -> p c k", c=1).broadcast_to(
                [128, TPC, 128])

            # ---- W_out: interleave PE chunks and DVE chunks on the sync ring ----
            out_dve = cpool.tile([128, NVT_DVE], f32)
            wt_tiles = []
            for ch in range(N_WT):
                w_sb = wtpool.tile([128, VCHUNK], f32)
                nc.sync.dma_start(
                    w_sb[:], wt_d.ap()[:, ch * VCHUNK:(ch + 1) * VCHUNK])
                wt_tiles.append(w_sb)
                if ch < N_WN:
                    wn_sb = wnpool.tile([128, TPC, 128], f32)
                    nc.sync.dma_start(
                        wn_sb[:],
                        wn_d.ap()[ch * VCHUNK:(ch + 1) * VCHUNK, :]
                        .rearrange("(c p) k -> p c k", p=128))
                    mscr = spool.tile([128, VCHUNK], f32, tag="scr")
                    m3 = mscr[:].rearrange("p (c k) -> p c k", k=128)
                    nc.vector.tensor_mul(m3, wn_sb[:], hb_bc)
                    nc.vector.tensor_reduce(
                        out_dve[:, ch * TPC:(ch + 1) * TPC], m3,
                        mybir.AxisListType.X, ALU.add)

            for pair in range(N_PAIR):
                w_sb = wt_tiles[pair // 5]
                base = (pair % 5) * 1024
                ps = pspool.tile([1, 1024], f32, tag="wps")
                nc.tensor.matmul(ps[:, 0:512], h_col[:],
                                 w_sb[:, base:base + 512], start=True, stop=True)
                nc.tensor.matmul(ps[:, 512:1024], h_col[:],
                                 w_sb[:, base + 512:base + 1024],
                                 start=True, stop=True)
                row = epool.tile([1, 1024], f32, tag="evac")
                nc.scalar.activation(row[:], ps[:], AF.Copy)
                nc.gpsimd.dma_start(ope_d.ap()[pair:pair + 1, :], row[:])

            nc.sync.dma_start(opd_d.ap(), out_dve[:])

    nc.finalize()
    return nc


def _prep_in_maps(input, hidden, cell, W_ref, b_ref, emb, W_ih, W_hh, b_ih,
                  b_hh, W_out, b_out):
    tok = int(np.asarray(input).reshape(-1)[0])
    x_row = np.asarray(emb[tok], dtype=np.float32).reshape(H)
    hidden_flat = np.asarray(hidden, dtype=np.float32).reshape(-1)
    cell_flat = np.asarray(cell, dtype=np.float32).reshape(-1)
    W_ref = np.asarray(W_ref, dtype=np.float32)
    b_ref = np.asarray(b_ref, dtype=np.float32)
    W_ih = np.asarray(W_ih, dtype=np.float32)
    W_hh = np.asarray(W_hh, dtype=np.float32)
    b_ih = np.asarray(b_ih, dtype=np.float32)
    b_hh = np.asarray(b_hh, dtype=np.float32)
    W_out = np.asarray(W_out, dtype=np.float32)
    KH = W_ref.shape[1]  # 1028

    W_hh_eff = W_hh @ W_ref                      # [4H, KH]
    bias_eff = b_ih + b_hh + W_hh @ b_ref        # [4H]

    z_raw = np.zeros(KZ, np.float32)
    z_raw[:H] = x_row
    z_raw[H:H + KH] = hidden_flat
    z_host = np.ascontiguousarray(z_raw.reshape(NT_Z, 128).T)     # [128,17]

    cz_raw = np.zeros(KC, np.float32)
    cz_raw[:KH] = cell_flat
    cz_host = np.ascontiguousarray(cz_raw.reshape(NT_C, 128).T)   # [128,9]

    WT = np.ascontiguousarray(W_out.T)           # [H, V]

    in_maps = []
    for m in range(N_CORES):
        sl = slice(128 * m, 128 * (m + 1))
        rows = (np.arange(4)[:, None] * H + 128 * m + np.arange(128)[None, :]
                ).reshape(-1)                    # [512]
        G = np.zeros((KZ, 512), np.float32)
        G[:H] = W_ih[rows].T
        G[H:H + KH] = W_hh_eff[rows].T
        Ac = np.zeros((KC, 128), np.float32)
        Ac[:KH] = W_ref[sl].T
        bias_m = np.ascontiguousarray(bias_eff[rows].reshape(1, 512))
        bref_m = np.ascontiguousarray(b_ref[sl].reshape(1, 128))
        Wt = np.zeros((128, VP_PE), np.float32)
        Wt[:, :VP_PE] = WT[sl, :VP_PE]
        Wn = np.zeros((VP_DVE, 128), np.float32)
        Wn[:V - VP_PE] = W_out[VP_PE:, sl]
        in_maps.append({
            "z": z_host, "cz": cz_host, "g_w": G, "ac_w": Ac,
            "bias": bias_m, "bref": bref_m, "wt": Wt, "wn": Wn,
        })
    return in_maps


def _assemble(results, b_out):
    b_out = np.asarray(b_out, dtype=np.float32)
    pe = np.zeros(VP_PE, np.float32)
    dve = np.zeros((128, NVT_DVE), np.float32)
    h_parts, c_parts = [], []
    for r in results:
        pe += r["out_pe"].reshape(-1)
        dve += r["out_dve"]
        h_parts.append(r["out_h"].reshape(128))
        c_parts.append(r["out_c"].reshape(128))
    logits = np.empty(VPAD, np.float32)
    logits[:VP_PE] = pe
    logits[VP_PE:] = np.ascontiguousarray(dve.T).reshape(-1)
    logits = logits[:V] + b_out
    output = logits.reshape(1, V).astype(np.float32)
    h_new = np.concatenate(h_parts).reshape(1, 1, H).astype(np.float32)
    c_new = np.concatenate(c_parts).reshape(1, 1, H).astype(np.float32)
    return output, h_new, c_new


def _kernel_impl(inputs, trace=False, trace_cores=None):
    from concourse.bass_utils import run_bass_kernel_spmd

    if "nc" not in _cache:
        _cache["nc"] = _build_bass()
    nc = _cache["nc"]
    in_maps = _prep_in_maps(**inputs)
    res = run_bass_kernel_spmd(nc, in_maps, list(range(N_CORES)),
                               trace=trace, trace_cores=trace_cores)
    outs = _assemble(res.results, inputs["b_out"])
    return outs, res


def kernel(**inputs):
    outs, _ = _kernel_impl(inputs, trace=False)
    return outs
